# revision 6
# baseline (speedup 1.0000x reference)
"""Jamba sparse-MoE block on 8 Trainium2 NeuronCores (expert-parallel).

Strategy
--------
- Routing (router matmul + softmax + top-2) is computed with jax on the host
  using the exact op sequence of the reference so expert selection matches
  bit-for-bit (one token in the dataset has a top2/top3 probability gap of
  ~5e-7; any rounding difference there would flip its expert assignment).
- Tokens are dispatched (gathered) per expert on the host; core e runs the
  dense gate/up/silu/mul/down FFN of expert e over its ~2k assigned tokens.
  This is the "all-to-all dispatch by top_k_index + expert-parallel weights"
  sharding, with the dispatch done at input-sharding time.
- Each core's Bass kernel is PE-bound and runs matmuls as float32r (full PE
  rate at N>=256, ~1.5e-4 relative rounding) with fp32 PSUM accumulation:
    phase A: hid = silu(x @ gw.T) * (x @ uw.T)   [F x C], staged to DRAM
    phase B: y = (w_token * (hid.T @ dw.T))      [C x H]
- Outputs are scatter-added back into the full [T, H] buffer on the host
  (each token appears in exactly two experts' outputs).
"""

import math
import numpy as np
from contextlib import ExitStack

B, S, H, F, E, TOP_K = 4, 2048, 1024, 4096, 8, 2
T = B * S
N_CORES = 8
P = 128
HC = H // P  # 8 h-chunks
FB = F // P  # 32 f-blocks


def _token_tiles(C):
    assert C % 256 == 0 and C >= 256
    tiles = [512] * (C // 512)
    if C % 512:
        tiles.append(C % 512)
    return tiles


_PROGRAM_CACHE = {}


def _build_program(C, H_=H, F_=F, act="Silu"):
    """SPMD program for one expert's FFN over C token slots."""
    key = (C, H_, F_, act)
    if key in _PROGRAM_CACHE:
        return _PROGRAM_CACHE[key]
    import concourse.bacc as bacc
    import concourse.mybir as mybir
    import concourse.tile as tile

    HC = H_ // P
    FB = F_ // P
    HH = H_ // 2  # phase-B H half width
    f32 = mybir.dt.float32
    f32r = mybir.dt.float32r
    AF = mybir.ActivationFunctionType
    NT128 = C // P
    tiles = _token_tiles(C)

    nc = bacc.Bacc("TRN2", target_bir_lowering=False, debug=False, num_devices=N_CORES)

    x_d = nc.dram_tensor("x", [P, HC, C], f32r, kind="ExternalInput")
    gw_d = nc.dram_tensor("gw", [FB, P, HC, P], f32r, kind="ExternalInput")
    uw_d = nc.dram_tensor("uw", [FB, P, HC, P], f32r, kind="ExternalInput")
    dw_d = nc.dram_tensor("dw", [P, FB, H_], f32r, kind="ExternalInput")
    wt_d = nc.dram_tensor("wt", [NT128, P], f32, kind="ExternalInput")
    y_d = nc.dram_tensor("y", [NT128, P, H_], f32, kind="ExternalOutput")
    hid_d = nc.dram_tensor("hid", [NT128, FB, P, P], f32r)  # internal staging

    with tile.TileContext(nc) as tc:
        with ExitStack() as ctx:
            xpool = ctx.enter_context(tc.tile_pool(name="xp", bufs=1))
            wtpool = ctx.enter_context(tc.tile_pool(name="wtp", bufs=1))
            gwpool = ctx.enter_context(tc.tile_pool(name="gwp", bufs=2))
            uwpool = ctx.enter_context(tc.tile_pool(name="uwp", bufs=2))
            sgpool = ctx.enter_context(tc.tile_pool(name="sgp", bufs=2))
            hspool = ctx.enter_context(tc.tile_pool(name="hsp", bufs=3))
            dwpool = ctx.enter_context(tc.tile_pool(name="dwp", bufs=1))
            htpool = ctx.enter_context(tc.tile_pool(name="htp", bufs=2))
            ypool = ctx.enter_context(tc.tile_pool(name="yp", bufs=2))
            psa = ctx.enter_context(tc.tile_pool(name="psa", bufs=2, space="PSUM"))
            psb = ctx.enter_context(tc.tile_pool(name="psb", bufs=2, space="PSUM"))

            x_t = xpool.tile([P, HC, C], f32r)
            nc.sync.dma_start(x_t[:], x_d.ap())
            wt_t = wtpool.tile([P, NT128], f32)
            nc.sync.dma_start(wt_t[:], wt_d.ap().rearrange("n p -> p n"))

            # ---- Phase A: hid[f, t] = silu(g) * u, staged to DRAM ----
            for fb in range(FB):
                gw_t = gwpool.tile([P, HC, P], f32r)
                nc.sync.dma_start(gw_t[:], gw_d.ap()[fb])
                uw_t = uwpool.tile([P, HC, P], f32r)
                nc.sync.dma_start(uw_t[:], uw_d.ap()[fb])
                t0 = 0
                for nt in tiles:
                    ps_g = psa.tile([P, 512], f32, name="ps_g")[:, :nt]
                    ps_u = psa.tile([P, 512], f32, name="ps_u")[:, :nt]
                    for hc in range(HC):
                        nc.tensor.matmul(
                            ps_g,
                            gw_t[:, hc, :],
                            x_t[:, hc, t0 : t0 + nt],
                            start=(hc == 0),
                            stop=(hc == HC - 1),
                        )
                    for hc in range(HC):
                        nc.tensor.matmul(
                            ps_u,
                            uw_t[:, hc, :],
                            x_t[:, hc, t0 : t0 + nt],
                            start=(hc == 0),
                            stop=(hc == HC - 1),
                        )
                    sg = sgpool.tile([P, 512], f32, name="sg")[:, :nt]
                    nc.scalar.activation(sg, ps_g, getattr(AF, act))
                    hid_sb = hspool.tile([P, 512], f32r, name="hid_sb")[:, :nt]
                    nc.vector.tensor_mul(hid_sb, sg, ps_u)
                    ts0 = t0 // P
                    k = nt // P
                    nc.sync.dma_start(
                        hid_d.ap()[ts0 : ts0 + k, fb].rearrange("s f t -> f s t"),
                        hid_sb.rearrange("p (s t) -> p s t", s=k),
                    )
                    t0 += nt

            # ---- Phase B: y[t, :] = w[t] * (hid[:, t].T @ dw.T), H halves ----
            for nh in range(2):
                dwc = dwpool.tile([P, FB, HH], f32r, name="dwc")
                nc.sync.dma_start(dwc[:], dw_d.ap()[:, :, nh * HH : (nh + 1) * HH])
                for tt in range(NT128):
                    hid_t = htpool.tile([P, FB, P], f32r, name="hid_t")
                    nc.sync.dma_start(
                        hid_t[:], hid_d.ap()[tt].rearrange("b f t -> f b t")
                    )
                    ps_y = psb.tile([P, HH], f32, name="ps_y")
                    for fb in range(FB):
                        nc.tensor.matmul(
                            ps_y[:],
                            hid_t[:, fb, :],
                            dwc[:, fb, :],
                            start=(fb == 0),
                            stop=(fb == FB - 1),
                        )
                    y_sb = ypool.tile([P, HH], f32, name="y_sb")
                    nc.scalar.activation(
                        y_sb[:], ps_y[:], AF.Copy, scale=wt_t[:, tt : tt + 1]
                    )
                    nc.sync.dma_start(
                        y_d.ap()[tt, :, nh * HH : (nh + 1) * HH], y_sb[:]
                    )
    nc.compile()
    _PROGRAM_CACHE[key] = nc
    return nc


def _routing(hidden_states, router_w):
    """Replicate the reference's routing ops exactly (same jax ops, default
    platform) so top-2 selection matches bit-for-bit."""
    import jax
    import jax.numpy as jnp

    x = jnp.asarray(hidden_states).reshape(-1, H)
    router_logits = x @ jnp.asarray(router_w).T
    routing_weights = jax.nn.softmax(router_logits.astype(jnp.float32), axis=-1)
    top_k_weights, top_k_index = jax.lax.top_k(routing_weights, TOP_K)
    return np.asarray(top_k_index), np.asarray(top_k_weights, dtype=np.float32)


def kernel(hidden_states, router_w, gate_w, up_w, down_w):
    from concourse.bass_utils import run_bass_kernel_spmd

    hidden_states = np.asarray(hidden_states, dtype=np.float32)
    router_w = np.asarray(router_w, dtype=np.float32)
    gate_w = np.asarray(gate_w, dtype=np.float32)
    up_w = np.asarray(up_w, dtype=np.float32)
    down_w = np.asarray(down_w, dtype=np.float32)

    tki, tkw = _routing(hidden_states, router_w)
    xf = hidden_states.reshape(T, H)

    idx_list, w_list = [], []
    for e in range(E):
        sel = tki == e  # [T, 2]
        tok = sel.any(axis=1)
        idx = np.nonzero(tok)[0]
        w = np.where(sel[:, 0], tkw[:, 0], tkw[:, 1])[idx]
        idx_list.append(idx)
        w_list.append(w.astype(np.float32))

    max_ne = max(len(i) for i in idx_list)
    C = max(512, int(math.ceil(max_ne / 256.0)) * 256)
    NT128 = C // P

    nc = _build_program(C)

    in_maps = []
    for e in range(E):
        idx, w = idx_list[e], w_list[e]
        ne = len(idx)
        xg = np.zeros((C, H), np.float32)
        xg[:ne] = xf[idx]
        wp = np.zeros((C,), np.float32)
        wp[:ne] = w
        in_maps.append(
            {
                "x": np.ascontiguousarray(
                    xg.T.reshape(HC, P, C).transpose(1, 0, 2)
                ),
                "gw": np.ascontiguousarray(
                    gate_w[e].reshape(FB, P, HC, P).transpose(0, 3, 2, 1)
                ),
                "uw": np.ascontiguousarray(
                    up_w[e].reshape(FB, P, HC, P).transpose(0, 3, 2, 1)
                ),
                "dw": np.ascontiguousarray(
                    down_w[e].T.reshape(FB, P, H).transpose(1, 0, 2)
                ),
                "wt": np.ascontiguousarray(wp.reshape(NT128, P)),
            }
        )

    res = run_bass_kernel_spmd(nc, in_maps, core_ids=list(range(N_CORES)))

    out = np.zeros((T, H), np.float32)
    for e in range(E):
        idx = idx_list[e]
        y = res.results[e]["y"].reshape(C, H)
        out[idx] += y[: len(idx)]
    return out.reshape(B, S, H)
